# revision 24
# baseline (speedup 1.0000x reference)
"""Trainium2 Bass kernel for nn_BatchRankingMSE_Loss (N=8192, 8 cores).

Reformulation: sort by labels on host (a pure data permutation). With q =
label-sorted preds, define for every pair a<b (sorted positions) the strict
indicator X(a,b) = 1{q_b < q_a + M}. Then
  ranking          = M*TOT + sum_a q_a*rows_a - sum_b q_b*cols_b
  grad_a (ranking) = rows_a - cols_a,   TOT = sum(X)
with rows/cols the row/column sums of X. The device evaluates the X grid
once and reduces it both ways:

Per core (SPMD, identical program; core c owns row-tiles
R_c = {8k + (c+k)%8}): 8 big flip-layout ops, op k =
[128 partitions = a-values of tile R_c[k]] x [free b in [128*(8k+1), 8192)].
Each op is split into DVE pieces (tensor_scalar is_lt with add-reduce
accum) and ACT pieces (Sign activation with accum), sub-split at the qj
half-DMA boundary so compute starts on the first half:
  - accum_out (free-axis sum)  -> row sums
  - the out tiles, streamed through TensorE with one-hot stationary columns
    into a single [16, 512] PSUM tile -> column sums (chunk m of 512 b's
    accumulates into PSUM row m; ACT sign tiles use 0.5-valued stationary)
Zero-stationary warmup matmuls keep the PE HAM busy during the DMA wait.
Window overshoot (b at/below own position) and the 8 uncovered diagonal
tiles {8k} are corrected exactly on host; mse partials also on device.
"""

import numpy as np

MARGIN = 2.0
EPS = 1e-4
N = 8192
NCORES = 8
RPC = N // NCORES        # rows per core = 1024
ACT_ENABLE = True
HALF = 4096              # qj DMA half boundary

_CACHE = {}
LAST_RESULTS = None      # test.py introspects timing from here


# ---------------------------------------------------------------- plan ----
def _core_rowtiles(c):
    return [8 * k + (c + k) % 8 for k in range(8)]


WSTART = [128 * (8 * k + 1) for k in range(8)]     # flip-op window starts


def _make_pieces(act_enable=ACT_ENABLE):
    """Partition each op-k window into engine pieces (uniform across cores).

    Returns list of (k, engine, lo, hi) with 512-aligned boundaries, also
    split at HALF so early pieces only need the first qj half.
    """
    cV = lambda fd: 480 + 1.042 * fd
    cA = lambda fd: 700 + 0.833 * fd
    dve = 2000.0          # mse + psum copy + slack
    act = 1400.0          # Sign table load
    pieces = []
    # choose ACT suffix split per op by greedy balance
    for k in sorted(range(8), key=lambda k: WSTART[k]):
        w = WSTART[k]
        fd = N - w
        best = None
        if not act_enable:
            best = (0, N, dve + cV(fd), act)
        else:
            for s in range(w, N + 1, 512):
                fv, fa = s - w, N - s
                d2 = dve + (cV(fv) if fv else 0)
                a2 = act + (cA(fa) if fa else 0)
                m = max(d2, a2)
                if best is None or m < best[0]:
                    best = (m, s, d2, a2)
        _, s, dve, act = best
        for (eng, lo, hi) in (("V", w, s), ("A", s, N)):
            if lo >= hi:
                continue
            cuts = [b for b in (2048, HALF) if lo < b < hi]
            for a, b in zip([lo] + cuts, cuts + [hi]):
                pieces.append((k, eng, a, b))
    # qj slices arrive high-first: order by descending start quarter,
    # then size desc
    pieces.sort(key=lambda p: (-(p[2] // 2048), -(p[3] - p[2])))
    return pieces


PIECES = _make_pieces()
NP_ = len(PIECES)


# ------------------------------------------------------------- program ----
def build_nc():
    import concourse.bass as bass
    import concourse.mybir as mybir
    from concourse import bacc, tile

    dt = mybir.dt
    Af = mybir.ActivationFunctionType
    Op = mybir.AluOpType

    nc = bacc.Bacc(None)
    qj_in = nc.dram_tensor("qj", [N], dt.float16, kind="ExternalInput")
    qip_in = nc.dram_tensor("qip", [128, 8], dt.float32, kind="ExternalInput")
    stoh_in = nc.dram_tensor("stoh", [544], dt.float16, kind="ExternalInput")
    prow_in = nc.dram_tensor("prow", [128, 8], dt.float32, kind="ExternalInput")
    lrow_in = nc.dram_tensor("lrow", [128, 8], dt.float32, kind="ExternalInput")
    acc_out = nc.dram_tensor("acc", [128, NP_], dt.float32, kind="ExternalOutput")
    cols_out = nc.dram_tensor("colsum", [16, 512], dt.float32, kind="ExternalOutput")
    mse_out = nc.dram_tensor("msesq", [128, 1], dt.float32, kind="ExternalOutput")

    # PE chunk-matmuls per piece: (piece idx, chunk m, lo, hi)
    mms = []
    for pi, (k, eng, plo, phi) in enumerate(PIECES):
        for m in range(plo // 512, (phi + 511) // 512):
            lo, hi = max(plo, 512 * m), min(phi, 512 * (m + 1))
            if lo < hi:
                mms.append((pi, m, lo, hi))
    NWARM = 5

    with tile.TileContext(nc) as tc:
        with (
            tc.tile_pool(name="persist", bufs=1) as pp,
            tc.tile_pool(name="work", bufs=4) as wp,
            tc.tile_pool(name="psum", bufs=1, space="PSUM") as qp,
        ):
            qj = pp.tile([128, N], dt.float16)
            qip = pp.tile([128, 8], dt.float32)
            stoh = pp.tile([128, 544], dt.float16)
            acc = pp.tile([128, NP_], dt.float32)
            msea = pp.tile([128, 1], dt.float32)
            csb = pp.tile([16, 512], dt.float32)
            pr = pp.tile([128, 8], dt.float32)
            lr = pp.tile([128, 8], dt.float32)
            dmse = pp.tile([128, 8], dt.float32)
            sqms = pp.tile([128, 8], dt.float32)

            psC = qp.tile([16, 512], dt.float32, tag="psc", name="psc")

            # stoh/qip first (warmup + piece deps), then qj top-half first
            # (nearly all window work needs high b); mse inputs off-queue
            nc.sync.dma_start(stoh[:], stoh_in[:].partition_broadcast(128))
            nc.sync.dma_start(qip[:], qip_in[:])
            for s in (3, 2, 1, 0):
                cs = slice(s * 2048, (s + 1) * 2048)
                nc.sync.dma_start(qj[:, cs], qj_in[cs].partition_broadcast(128))
            nc.scalar.dma_start(pr[:], prow_in[:])
            nc.scalar.dma_start(lr[:], lrow_in[:])

            # load the Sign table while DMAs stream (dummy op on stoh)
            dumm = pp.tile([128, 16], dt.float16)
            nc.scalar.activation(dumm[:], stoh[:, 0:16],
                                 Af.Sign, bias=0.0, scale=1.0)

            # PE warmup: zero-stationary matmuls (add 0 into psC) to lift
            # the HAM clock gate while qj streams in. First one clears psC.
            for wi in range(NWARM):
                nc.tensor.matmul(psC[0:16, 0:512], stoh[:, 512:528],
                                 stoh[:, 0:512], start=(wi == 0), stop=False)

            # mse partials: sum_free (p-l)^2 per partition
            nc.vector.scalar_tensor_tensor(
                dmse[:], pr[:], 0.0, lr[:], op0=Op.add, op1=Op.subtract)
            nc.vector.scalar_tensor_tensor(
                sqms[:], dmse[:], 1.0, dmse[:], op0=Op.mult, op1=Op.mult,
                accum_out=msea[:])
            nc.sync.dma_start(mse_out[:], msea[:])

            for pi, (k, eng, plo, phi) in enumerate(PIECES):
                fd = phi - plo
                t = wp.tile([128, fd], dt.float16, tag=eng)
                if eng == "V":
                    # X = 1{q_b < q_a + M}; op1/scalar2 = add-reduce to accum
                    nc.vector.tensor_scalar(
                        t[:], qj[:, plo:phi], qip[:, k:k + 1], 0.0,
                        op0=Op.is_lt, op1=Op.add,
                        accum_out=acc[:, pi:pi + 1])
                else:
                    nc.scalar.activation(
                        t[:], qj[:, plo:phi], Af.Sign, bias=qip[:, k:k + 1],
                        scale=-1.0, accum_out=acc[:, pi:pi + 1])
                last = (pi == NP_ - 1)
                for (pj, m, lo, hi) in mms:
                    if pj != pi:
                        continue
                    sv = 16 * m + (256 if eng == "A" else 0)
                    nc.tensor.matmul(
                        psC[0:16, lo - 512 * m:hi - 512 * m],
                        stoh[:, sv:sv + 16], t[:, lo - plo:hi - plo],
                        start=False,
                        stop=(last and (pj, m, lo, hi) == mms[-1]))

            nc.vector.tensor_copy(csb[:], psC[:])
            nc.sync.dma_start(cols_out[:], csb[:])
            nc.scalar.dma_start(acc_out[:], acc[:])
    if not nc.is_finalized():
        nc.finalize()
    return nc


# ---------------------------------------------------------- host side ----
def _sorted_q(preds, labels):
    labels32 = np.asarray(labels, dtype=np.float32)
    perm = np.argsort(labels32, kind="stable")
    q16 = np.asarray(preds, dtype=np.float32)[perm].astype(np.float16)
    return q16, q16.astype(np.float64)


def make_in_maps(preds, labels):
    preds = np.asarray(preds, dtype=np.float32)
    labels = np.asarray(labels, dtype=np.float32)
    q16, qd = _sorted_q(preds, labels)
    stoh = np.zeros(544, dtype=np.float16)
    for m in range(16):
        stoh[16 * m + m] = 1.0          # DVE chunks: weight 1.0
        stoh[256 + 16 * m + m] = 0.5    # ACT sign chunks: weight 0.5
    # stoh[512:544] stays 0: zero-stationary for PE warmup
    in_maps = []
    for c in range(NCORES):
        R = _core_rowtiles(c)
        i_of_m = np.concatenate([128 * r + np.arange(128) for r in R])
        qip = np.ascontiguousarray(
            (qd[i_of_m] + MARGIN).reshape(8, 128).T.astype(np.float32))
        rows = slice(c * RPC, (c + 1) * RPC)
        in_maps.append({
            "qj": q16,
            "qip": qip,
            "stoh": stoh,
            "prow": np.ascontiguousarray(preds[rows].reshape(8, 128).T),
            "lrow": np.ascontiguousarray(labels[rows].reshape(8, 128).T),
        })
    return in_maps


def combine(results, preds, labels):
    """Fold device partials into the scalar loss (host, f64, exact)."""
    preds64 = np.asarray(preds, dtype=np.float64)
    labels64 = np.asarray(labels, dtype=np.float64)
    _, qd = _sorted_q(preds, labels)

    rows = np.zeros(N)
    cols = np.zeros(N)
    msesum = 0.0
    for c in range(NCORES):
        res = results[c]
        R = _core_rowtiles(c)
        acc = res["acc"].astype(np.float64)
        colsum = res["colsum"].astype(np.float64)
        msesum += float(res["msesq"].astype(np.float64).sum())

        # cols decode: cell [m, off] <-> b = 512m + off
        colsc = colsum.reshape(-1).copy()
        colsc[:128] = 0.0                          # b < 128: never covered
        nact = np.zeros(N)
        for (k, eng, plo, phi) in PIECES:
            if eng == "A":
                nact[plo:phi] += 64.0              # sign tiles wrote X - 0.5
        colsc[128:] += nact[128:]
        cols += colsc

        for k in range(8):
            r = R[k]
            w = WSTART[k]
            apos = 128 * r + np.arange(128)
            qa = qd[apos]
            radd = np.zeros(128)
            for pi, (kk, eng, plo, phi) in enumerate(PIECES):
                if kk != k:
                    continue
                if eng == "V":
                    radd += acc[:, pi]
                else:
                    radd += (acc[:, pi] + (phi - plo)) / 2.0
            # pollution: device also counted b with pos(b) <= pos(a)
            hi = 128 * (r + 1)
            if hi > w:
                win = np.arange(w, hi)
                qb = qd[win]
                lt = (qb[None, :] < qa[:, None] + MARGIN)
                eq = (qb[None, :] == qa[:, None] + MARGIN)
                posmask = (win[None, :] <= apos[:, None])
                actseg = np.zeros(hi - w, dtype=bool)
                for (kk, eng, plo, phi) in PIECES:
                    if kk == k and eng == "A":
                        lo_i, hi_i = max(plo - w, 0), min(phi, hi) - w
                        if hi_i > lo_i:
                            actseg[lo_i:hi_i] = True
                dveseg = ~actseg
                pv = (lt & posmask & dveseg[None, :]).sum(1)
                pa = ((lt & posmask & actseg[None, :]).sum(1)
                      + 0.5 * (eq & posmask & actseg[None, :]).sum(1))
                radd = radd - pv - pa
                cv = (lt & posmask & dveseg[None, :]).sum(0)
                ca = ((lt & posmask & actseg[None, :]).sum(0)
                      + 0.5 * (eq & posmask & actseg[None, :]).sum(0))
                np.add.at(cols, win, -(cv + ca))
            rows[apos] += radd

    # host-exact diagonal tiles {8k} (not covered by any window)
    for t in range(0, 64, 8):
        qa = qd[128 * t:128 * (t + 1)]
        X = (qa[None, :] < qa[:, None] + MARGIN)
        X &= np.triu(np.ones((128, 128), dtype=bool), k=1)
        rows[128 * t:128 * (t + 1)] += X.sum(1)
        cols[128 * t:128 * (t + 1)] += X.sum(0)

    grad = rows - cols
    TOT = rows.sum()
    ranking = MARGIN * TOT + qd @ grad
    g2 = np.sqrt((grad * grad).sum())
    mse = msesum / N
    g1 = 2.0 * np.sqrt(msesum) / N
    return np.float32(mse + g1 / (g2 + EPS) * ranking)


# ------------------------------------------------- numpy device model ----
def _sim_outputs(preds, labels):
    """Produce the same outputs the device would (for offline validation)."""
    preds = np.asarray(preds, dtype=np.float32)
    labels = np.asarray(labels, dtype=np.float32)
    q16, qd = _sorted_q(preds, labels)
    out = []
    for c in range(NCORES):
        R = _core_rowtiles(c)
        acc = np.zeros((128, NP_))
        colsum = np.zeros((16, 512))
        for pi, (k, eng, plo, phi) in enumerate(PIECES):
            r = R[k]
            qa = qd[128 * r:128 * (r + 1)]
            if eng == "V":
                X = (qd[None, plo:phi] < qa[:, None] + MARGIN).astype(np.float64)
                acc[:, pi] = X.sum(1)
                wgt, T = 1.0, X
            else:
                sgn = np.sign(qa[:, None] + MARGIN - qd[None, plo:phi])
                acc[:, pi] = sgn.sum(1)
                wgt, T = 0.5, sgn
            for m in range(plo // 512, (phi + 511) // 512):
                lo, hi = max(plo, 512 * m), min(phi, 512 * (m + 1))
                if lo < hi:
                    colsum[m, lo - 512 * m:hi - 512 * m] += \
                        wgt * T[:, lo - plo:hi - plo].sum(0)
        rows = slice(c * RPC, (c + 1) * RPC)
        d = preds[rows].astype(np.float64) - labels[rows].astype(np.float64)
        msesq = d.reshape(8, 128).T
        out.append({
            "acc": acc.astype(np.float32),
            "colsum": colsum.astype(np.float32),
            "msesq": (msesq * msesq).sum(1, keepdims=True).astype(np.float32),
        })
    return out


# ------------------------------------------------------------- driver ----
def kernel(preds, labels):
    global LAST_RESULTS
    from concourse.bass_utils import run_bass_kernel_spmd

    if "nc" not in _CACHE:
        _CACHE["nc"] = build_nc()
    in_maps = make_in_maps(preds, labels)
    res = run_bass_kernel_spmd(_CACHE["nc"], in_maps, list(range(NCORES)))
    LAST_RESULTS = res
    return combine(res.results, preds, labels)


# revision 29
# speedup vs baseline: 1.1571x; 1.1571x over previous
"""Trainium2 Bass kernel for nn_BatchRankingMSE_Loss (N=8192, 8 cores).

Reformulation: sort by labels on host (a pure data permutation). With q =
label-sorted preds, define for every pair a<b (sorted positions) the strict
indicator X(a,b) = 1{q_b < q_a + M}. Then
  ranking          = M*TOT + sum_a q_a*rows_a - sum_b q_b*cols_b
  grad_a (ranking) = rows_a - cols_a,   TOT = sum(X)
with rows/cols the row/column sums of X. The device evaluates the X grid
once and reduces it both ways:

Per core (SPMD, identical program; core c owns row-tiles
R_c = {8k + (c+k)%8}): 8 big flip-layout ops, op k =
[128 partitions = a-values of tile R_c[k]] x [free b in [128*(8k+1), 8192)].
Each op is split into DVE pieces (tensor_scalar is_lt with add-reduce
accum) and ACT pieces (Sign activation with accum), sub-split at the qj
half-DMA boundary so compute starts on the first half:
  - accum_out (free-axis sum)  -> row sums
  - the out tiles, streamed through TensorE with one-hot stationary columns
    into a single [16, 512] PSUM tile -> column sums (chunk m of 512 b's
    accumulates into PSUM row m; ACT sign tiles use 0.5-valued stationary)
Zero-stationary warmup matmuls keep the PE HAM busy during the DMA wait.
Window overshoot (b at/below own position) and the 8 uncovered diagonal
tiles {8k} are corrected exactly on host; mse partials also on device.
"""

import numpy as np

MARGIN = 2.0
EPS = 1e-4
N = 8192
NCORES = 8
RPC = N // NCORES        # rows per core = 1024
ACT_ENABLE = True
HALF = 4096              # qj DMA half boundary

_CACHE = {}
LAST_RESULTS = None      # test.py introspects timing from here


# ---------------------------------------------------------------- plan ----
def _core_rowtiles(c):
    return [8 * k + (c + k) % 8 for k in range(8)]


WSTART = [128 * (8 * k + 1) for k in range(8)]     # flip-op window starts


def _make_pieces(act_enable=ACT_ENABLE):
    """Partition each op-k window into engine pieces (uniform across cores).

    Returns list of (k, engine, lo, hi) with 512-aligned boundaries, also
    split at HALF so early pieces only need the first qj half.
    """
    cV = lambda fd: 430 + 1.042 * fd
    cA = lambda fd: 1150 + 0.833 * fd
    dve = 2000.0          # mse + psum copy + slack
    act = 1400.0          # Sign table load
    pieces = []
    # choose ACT suffix split per op by greedy balance
    for k in sorted(range(8), key=lambda k: WSTART[k]):
        w = WSTART[k]
        fd = N - w
        best = None
        if not act_enable:
            best = (0, N, dve + cV(fd), act)
        else:
            for s in range(w, N + 1, 512):
                fv, fa = s - w, N - s
                d2 = dve + (cV(fv) if fv else 0)
                a2 = act + (cA(fa) if fa else 0)
                m = max(d2, a2)
                if best is None or m < best[0]:
                    best = (m, s, d2, a2)
        _, s, dve, act = best
        for (eng, lo, hi) in (("V", w, s), ("A", s, N)):
            if lo >= hi:
                continue
            cuts = [b for b in (2048, HALF) if lo < b < hi]
            for a, b in zip([lo] + cuts, cuts + [hi]):
                pieces.append((k, eng, a, b))
    # order: by qj-slice arrival (quarter of the start), then size desc
    pieces.sort(key=lambda p: (p[2] // 2048, -(p[3] - p[2])))
    return pieces


PIECES = _make_pieces()
NP_ = len(PIECES)


# ------------------------------------------------------------- program ----
def build_nc():
    import concourse.bass as bass
    import concourse.mybir as mybir
    from concourse import bacc, tile

    dt = mybir.dt
    Af = mybir.ActivationFunctionType
    Op = mybir.AluOpType

    nc = bacc.Bacc(None)
    qj_in = nc.dram_tensor("qj", [N], dt.float16, kind="ExternalInput")
    qip_in = nc.dram_tensor("qip", [128, 8], dt.float32, kind="ExternalInput")
    stoh_in = nc.dram_tensor("stoh", [544], dt.float16, kind="ExternalInput")
    prow_in = nc.dram_tensor("prow", [128, 8], dt.float32, kind="ExternalInput")
    lrow_in = nc.dram_tensor("lrow", [128, 8], dt.float32, kind="ExternalInput")
    acc_out = nc.dram_tensor("acc", [128, NP_], dt.float32, kind="ExternalOutput")
    cols_out = nc.dram_tensor("colsum", [16, 512], dt.float32, kind="ExternalOutput")
    mse_out = nc.dram_tensor("msesq", [128, 1], dt.float32, kind="ExternalOutput")

    # PE chunk-matmuls per piece: (piece idx, chunk m, lo, hi)
    mms = []
    for pi, (k, eng, plo, phi) in enumerate(PIECES):
        for m in range(plo // 512, (phi + 511) // 512):
            lo, hi = max(plo, 512 * m), min(phi, 512 * (m + 1))
            if lo < hi:
                mms.append((pi, m, lo, hi))
    NWARM = 14

    with tile.TileContext(nc) as tc:
        with (
            tc.tile_pool(name="persist", bufs=1) as pp,
            tc.tile_pool(name="work", bufs=4) as wp,
            tc.tile_pool(name="psum", bufs=1, space="PSUM") as qp,
        ):
            qj = pp.tile([128, N], dt.float16)
            qip = pp.tile([128, 8], dt.float32)
            stoh = pp.tile([128, 544], dt.float16)
            acc = pp.tile([128, NP_], dt.float32)
            msea = pp.tile([128, 1], dt.float32)
            csb = pp.tile([16, 512], dt.float32)
            pr = pp.tile([128, 8], dt.float32)
            lr = pp.tile([128, 8], dt.float32)
            dmse = pp.tile([128, 8], dt.float32)
            sqms = pp.tile([128, 8], dt.float32)

            psC = qp.tile([16, 512], dt.float32, tag="psc", name="psc")

            nc.sync.dma_start(stoh[:], stoh_in[:].partition_broadcast(128))
            nc.sync.dma_start(qip[:], qip_in[:])
            for s in range(4):
                cs = slice(s * 2048, (s + 1) * 2048)
                nc.sync.dma_start(qj[:, cs], qj_in[cs].partition_broadcast(128))
            nc.sync.dma_start(pr[:], prow_in[:])
            nc.sync.dma_start(lr[:], lrow_in[:])

            # load the Sign table while DMAs stream (dummy op on stoh)
            dumm = pp.tile([128, 16], dt.float16)
            nc.scalar.activation(dumm[:], stoh[:, 0:16],
                                 Af.Sign, bias=0.0, scale=1.0)

            # PE warmup: zero-stationary matmuls (add 0 into psC) to lift
            # the HAM clock gate while qj streams in. First one clears psC.
            for wi in range(NWARM):
                nc.tensor.matmul(psC[0:16, 0:512], stoh[:, 512:528],
                                 stoh[:, 0:512], start=(wi == 0), stop=False)

            # mse partials: sum_free (p-l)^2 per partition
            nc.vector.scalar_tensor_tensor(
                dmse[:], pr[:], 0.0, lr[:], op0=Op.add, op1=Op.subtract)
            nc.vector.scalar_tensor_tensor(
                sqms[:], dmse[:], 1.0, dmse[:], op0=Op.mult, op1=Op.mult,
                accum_out=msea[:])
            nc.sync.dma_start(mse_out[:], msea[:])

            for pi, (k, eng, plo, phi) in enumerate(PIECES):
                fd = phi - plo
                t = wp.tile([128, fd], dt.float16, tag=eng)
                if eng == "V":
                    # X = 1{q_b < q_a + M}; op1/scalar2 = add-reduce to accum
                    nc.vector.tensor_scalar(
                        t[:], qj[:, plo:phi], qip[:, k:k + 1], 0.0,
                        op0=Op.is_lt, op1=Op.add,
                        accum_out=acc[:, pi:pi + 1])
                else:
                    nc.scalar.activation(
                        t[:], qj[:, plo:phi], Af.Sign, bias=qip[:, k:k + 1],
                        scale=-1.0, accum_out=acc[:, pi:pi + 1])
                last = (pi == NP_ - 1)
                for (pj, m, lo, hi) in mms:
                    if pj != pi:
                        continue
                    sv = 16 * m + (256 if eng == "A" else 0)
                    nc.tensor.matmul(
                        psC[0:16, lo - 512 * m:hi - 512 * m],
                        stoh[:, sv:sv + 16], t[:, lo - plo:hi - plo],
                        start=False,
                        stop=(last and (pj, m, lo, hi) == mms[-1]))

            nc.vector.tensor_copy(csb[:], psC[:])
            nc.sync.dma_start(cols_out[:], csb[:])
            nc.sync.dma_start(acc_out[:], acc[:])
    if not nc.is_finalized():
        nc.finalize()
    return nc


# ---------------------------------------------------------- host side ----
def _sorted_q(preds, labels):
    labels32 = np.asarray(labels, dtype=np.float32)
    perm = np.argsort(labels32, kind="stable")
    q16 = np.asarray(preds, dtype=np.float32)[perm].astype(np.float16)
    return q16, q16.astype(np.float64)


def make_in_maps(preds, labels):
    preds = np.asarray(preds, dtype=np.float32)
    labels = np.asarray(labels, dtype=np.float32)
    q16, qd = _sorted_q(preds, labels)
    stoh = np.zeros(544, dtype=np.float16)
    for m in range(16):
        stoh[16 * m + m] = 1.0          # DVE chunks: weight 1.0
        stoh[256 + 16 * m + m] = 0.5    # ACT sign chunks: weight 0.5
    # stoh[512:544] stays 0: zero-stationary for PE warmup
    in_maps = []
    for c in range(NCORES):
        R = _core_rowtiles(c)
        i_of_m = np.concatenate([128 * r + np.arange(128) for r in R])
        qip = np.ascontiguousarray(
            (qd[i_of_m] + MARGIN).reshape(8, 128).T.astype(np.float32))
        rows = slice(c * RPC, (c + 1) * RPC)
        in_maps.append({
            "qj": q16,
            "qip": qip,
            "stoh": stoh,
            "prow": np.ascontiguousarray(preds[rows].reshape(8, 128).T),
            "lrow": np.ascontiguousarray(labels[rows].reshape(8, 128).T),
        })
    return in_maps


def combine(results, preds, labels):
    """Fold device partials into the scalar loss (host, f64, exact)."""
    preds64 = np.asarray(preds, dtype=np.float64)
    labels64 = np.asarray(labels, dtype=np.float64)
    _, qd = _sorted_q(preds, labels)

    rows = np.zeros(N)
    cols = np.zeros(N)
    msesum = 0.0
    for c in range(NCORES):
        res = results[c]
        R = _core_rowtiles(c)
        acc = res["acc"].astype(np.float64)
        colsum = res["colsum"].astype(np.float64)
        msesum += float(res["msesq"].astype(np.float64).sum())

        # cols decode: cell [m, off] <-> b = 512m + off
        colsc = colsum.reshape(-1).copy()
        colsc[:128] = 0.0                          # b < 128: never covered
        nact = np.zeros(N)
        for (k, eng, plo, phi) in PIECES:
            if eng == "A":
                nact[plo:phi] += 64.0              # sign tiles wrote X - 0.5
        colsc[128:] += nact[128:]
        cols += colsc

        for k in range(8):
            r = R[k]
            w = WSTART[k]
            apos = 128 * r + np.arange(128)
            qa = qd[apos]
            radd = np.zeros(128)
            for pi, (kk, eng, plo, phi) in enumerate(PIECES):
                if kk != k:
                    continue
                if eng == "V":
                    radd += acc[:, pi]
                else:
                    radd += (acc[:, pi] + (phi - plo)) / 2.0
            # pollution: device also counted b with pos(b) <= pos(a)
            hi = 128 * (r + 1)
            if hi > w:
                win = np.arange(w, hi)
                qb = qd[win]
                lt = (qb[None, :] < qa[:, None] + MARGIN)
                eq = (qb[None, :] == qa[:, None] + MARGIN)
                posmask = (win[None, :] <= apos[:, None])
                actseg = np.zeros(hi - w, dtype=bool)
                for (kk, eng, plo, phi) in PIECES:
                    if kk == k and eng == "A":
                        lo_i, hi_i = max(plo - w, 0), min(phi, hi) - w
                        if hi_i > lo_i:
                            actseg[lo_i:hi_i] = True
                dveseg = ~actseg
                pv = (lt & posmask & dveseg[None, :]).sum(1)
                pa = ((lt & posmask & actseg[None, :]).sum(1)
                      + 0.5 * (eq & posmask & actseg[None, :]).sum(1))
                radd = radd - pv - pa
                cv = (lt & posmask & dveseg[None, :]).sum(0)
                ca = ((lt & posmask & actseg[None, :]).sum(0)
                      + 0.5 * (eq & posmask & actseg[None, :]).sum(0))
                np.add.at(cols, win, -(cv + ca))
            rows[apos] += radd

    # host-exact diagonal tiles {8k} (not covered by any window)
    for t in range(0, 64, 8):
        qa = qd[128 * t:128 * (t + 1)]
        X = (qa[None, :] < qa[:, None] + MARGIN)
        X &= np.triu(np.ones((128, 128), dtype=bool), k=1)
        rows[128 * t:128 * (t + 1)] += X.sum(1)
        cols[128 * t:128 * (t + 1)] += X.sum(0)

    grad = rows - cols
    TOT = rows.sum()
    ranking = MARGIN * TOT + qd @ grad
    g2 = np.sqrt((grad * grad).sum())
    mse = msesum / N
    g1 = 2.0 * np.sqrt(msesum) / N
    return np.float32(mse + g1 / (g2 + EPS) * ranking)


# ------------------------------------------------- numpy device model ----
def _sim_outputs(preds, labels):
    """Produce the same outputs the device would (for offline validation)."""
    preds = np.asarray(preds, dtype=np.float32)
    labels = np.asarray(labels, dtype=np.float32)
    q16, qd = _sorted_q(preds, labels)
    out = []
    for c in range(NCORES):
        R = _core_rowtiles(c)
        acc = np.zeros((128, NP_))
        colsum = np.zeros((16, 512))
        for pi, (k, eng, plo, phi) in enumerate(PIECES):
            r = R[k]
            qa = qd[128 * r:128 * (r + 1)]
            if eng == "V":
                X = (qd[None, plo:phi] < qa[:, None] + MARGIN).astype(np.float64)
                acc[:, pi] = X.sum(1)
                wgt, T = 1.0, X
            else:
                sgn = np.sign(qa[:, None] + MARGIN - qd[None, plo:phi])
                acc[:, pi] = sgn.sum(1)
                wgt, T = 0.5, sgn
            for m in range(plo // 512, (phi + 511) // 512):
                lo, hi = max(plo, 512 * m), min(phi, 512 * (m + 1))
                if lo < hi:
                    colsum[m, lo - 512 * m:hi - 512 * m] += \
                        wgt * T[:, lo - plo:hi - plo].sum(0)
        rows = slice(c * RPC, (c + 1) * RPC)
        d = preds[rows].astype(np.float64) - labels[rows].astype(np.float64)
        msesq = d.reshape(8, 128).T
        out.append({
            "acc": acc.astype(np.float32),
            "colsum": colsum.astype(np.float32),
            "msesq": (msesq * msesq).sum(1, keepdims=True).astype(np.float32),
        })
    return out


# ------------------------------------------------------------- driver ----
def kernel(preds, labels):
    global LAST_RESULTS
    from concourse.bass_utils import run_bass_kernel_spmd

    if "nc" not in _CACHE:
        _CACHE["nc"] = build_nc()
    in_maps = make_in_maps(preds, labels)
    res = run_bass_kernel_spmd(_CACHE["nc"], in_maps, list(range(NCORES)))
    LAST_RESULTS = res
    return combine(res.results, preds, labels)


# revision 30
# speedup vs baseline: 1.1687x; 1.0100x over previous
"""Trainium2 Bass kernel for nn_BatchRankingMSE_Loss (N=8192, 8 cores).

Reformulation: sort by labels on host (a pure data permutation). With q =
label-sorted preds, define for every pair a<b (sorted positions) the strict
indicator X(a,b) = 1{q_b < q_a + M}. Then
  ranking          = M*TOT + sum_a q_a*rows_a - sum_b q_b*cols_b
  grad_a (ranking) = rows_a - cols_a,   TOT = sum(X)
with rows/cols the row/column sums of X. The device evaluates the X grid
once and reduces it both ways:

Per core (SPMD, identical program; core c owns row-tiles
R_c = {8k + (c+k)%8}): 8 big flip-layout ops, op k =
[128 partitions = a-values of tile R_c[k]] x [free b in [128*(8k+1), 8192)].
Each op is split into DVE pieces (tensor_scalar is_lt with add-reduce
accum) and ACT pieces (Sign activation with accum), sub-split at the qj
half-DMA boundary so compute starts on the first half:
  - accum_out (free-axis sum)  -> row sums
  - the out tiles, streamed through TensorE with one-hot stationary columns
    into a single [16, 512] PSUM tile -> column sums (chunk m of 512 b's
    accumulates into PSUM row m; ACT sign tiles use 0.5-valued stationary)
Zero-stationary warmup matmuls keep the PE HAM busy during the DMA wait.
Window overshoot (b at/below own position) and the 8 uncovered diagonal
tiles {8k} are corrected exactly on host; mse partials also on device.
"""

import numpy as np

MARGIN = 2.0
EPS = 1e-4
N = 8192
NCORES = 8
RPC = N // NCORES        # rows per core = 1024
ACT_ENABLE = True
HALF = 4096              # qj DMA half boundary

_CACHE = {}
LAST_RESULTS = None      # test.py introspects timing from here


# ---------------------------------------------------------------- plan ----
def _core_rowtiles(c):
    return [8 * k + (c + k) % 8 for k in range(8)]


WSTART = [128 * (8 * k + 1) for k in range(8)]     # flip-op window starts


def _make_pieces(act_enable=ACT_ENABLE):
    """Partition each op-k window into engine pieces (uniform across cores).

    Returns list of (k, engine, lo, hi) with 512-aligned boundaries, also
    split at HALF so early pieces only need the first qj half.
    """
    cV = lambda fd: 430 + 1.042 * fd
    cA = lambda fd: 1150 + 0.833 * fd
    dve = 2000.0          # mse + psum copy + slack
    act = 1400.0          # Sign table load
    pieces = []
    # choose ACT suffix split per op by greedy balance
    for k in sorted(range(8), key=lambda k: WSTART[k]):
        w = WSTART[k]
        fd = N - w
        best = None
        if not act_enable:
            best = (0, N, dve + cV(fd), act)
        else:
            for s in range(w, N + 1, 512):
                fv, fa = s - w, N - s
                d2 = dve + (cV(fv) if fv else 0)
                a2 = act + (cA(fa) if fa else 0)
                m = max(d2, a2)
                if best is None or m < best[0]:
                    best = (m, s, d2, a2)
        _, s, dve, act = best
        for (eng, lo, hi) in (("V", w, s), ("A", s, N)):
            if lo >= hi:
                continue
            cuts = [b for b in (2048, HALF) if lo < b < hi]
            for a, b in zip([lo] + cuts, cuts + [hi]):
                pieces.append((k, eng, a, b))
    # order: by qj-slice arrival (quarter of the start), then size desc
    pieces.sort(key=lambda p: (p[2] // 2048, -(p[3] - p[2])))
    return pieces


PIECES = _make_pieces()
NP_ = len(PIECES)


# ------------------------------------------------------------- program ----
def build_nc():
    import concourse.bass as bass
    import concourse.mybir as mybir
    from concourse import bacc, tile

    dt = mybir.dt
    Af = mybir.ActivationFunctionType
    Op = mybir.AluOpType

    nc = bacc.Bacc(None)
    qj_in = nc.dram_tensor("qj", [N], dt.float16, kind="ExternalInput")
    qip_in = nc.dram_tensor("qip", [128, 8], dt.float32, kind="ExternalInput")
    stoh_in = nc.dram_tensor("stoh", [544], dt.float16, kind="ExternalInput")
    prow_in = nc.dram_tensor("prow", [128, 8], dt.float32, kind="ExternalInput")
    lrow_in = nc.dram_tensor("lrow", [128, 8], dt.float32, kind="ExternalInput")
    acc_out = nc.dram_tensor("acc", [128, NP_], dt.float32, kind="ExternalOutput")
    cols_out = nc.dram_tensor("colsum", [16, 512], dt.float32, kind="ExternalOutput")
    mse_out = nc.dram_tensor("msesq", [128, 1], dt.float32, kind="ExternalOutput")

    # PE chunk-matmuls per piece: (piece idx, chunk m, lo, hi)
    mms = []
    for pi, (k, eng, plo, phi) in enumerate(PIECES):
        for m in range(plo // 512, (phi + 511) // 512):
            lo, hi = max(plo, 512 * m), min(phi, 512 * (m + 1))
            if lo < hi:
                mms.append((pi, m, lo, hi))
    NWARM = 14

    with tile.TileContext(nc) as tc:
        with (
            tc.tile_pool(name="persist", bufs=1) as pp,
            tc.tile_pool(name="work", bufs=4) as wp,
            tc.tile_pool(name="psum", bufs=1, space="PSUM") as qp,
        ):
            qj = pp.tile([128, N], dt.float16)
            qip = pp.tile([128, 8], dt.float32)
            stoh = pp.tile([128, 544], dt.float16)
            acc = pp.tile([128, NP_], dt.float32)
            msea = pp.tile([128, 1], dt.float32)
            csb = pp.tile([16, 512], dt.float32)
            pr = pp.tile([128, 8], dt.float32)
            lr = pp.tile([128, 8], dt.float32)
            dmse = pp.tile([128, 8], dt.float32)
            sqms = pp.tile([128, 8], dt.float32)

            psC = qp.tile([16, 512], dt.float32, tag="psc", name="psc")

            # qj owns the Sync queue from instruction zero; the small inputs
            # issue concurrently from the (still idle) Scalar queue
            for s in range(4):
                cs = slice(s * 2048, (s + 1) * 2048)
                nc.sync.dma_start(qj[:, cs], qj_in[cs].partition_broadcast(128))
            nc.scalar.dma_start(stoh[:], stoh_in[:].partition_broadcast(128))
            nc.scalar.dma_start(qip[:], qip_in[:])
            nc.scalar.dma_start(pr[:], prow_in[:])
            nc.scalar.dma_start(lr[:], lrow_in[:])

            # load the Sign table while DMAs stream (dummy op on stoh)
            dumm = pp.tile([128, 16], dt.float16)
            nc.scalar.activation(dumm[:], stoh[:, 0:16],
                                 Af.Sign, bias=0.0, scale=1.0)

            # PE warmup: zero-stationary matmuls (add 0 into psC) to lift
            # the HAM clock gate while qj streams in. First one clears psC.
            for wi in range(NWARM):
                nc.tensor.matmul(psC[0:16, 0:512], stoh[:, 512:528],
                                 stoh[:, 0:512], start=(wi == 0), stop=False)

            # mse partials: sum_free (p-l)^2 per partition
            nc.vector.scalar_tensor_tensor(
                dmse[:], pr[:], 0.0, lr[:], op0=Op.add, op1=Op.subtract)
            nc.vector.scalar_tensor_tensor(
                sqms[:], dmse[:], 1.0, dmse[:], op0=Op.mult, op1=Op.mult,
                accum_out=msea[:])
            nc.sync.dma_start(mse_out[:], msea[:])

            for pi, (k, eng, plo, phi) in enumerate(PIECES):
                fd = phi - plo
                t = wp.tile([128, fd], dt.float16, tag=eng)
                if eng == "V":
                    # X = 1{q_b < q_a + M}; op1/scalar2 = add-reduce to accum
                    nc.vector.tensor_scalar(
                        t[:], qj[:, plo:phi], qip[:, k:k + 1], 0.0,
                        op0=Op.is_lt, op1=Op.add,
                        accum_out=acc[:, pi:pi + 1])
                else:
                    nc.scalar.activation(
                        t[:], qj[:, plo:phi], Af.Sign, bias=qip[:, k:k + 1],
                        scale=-1.0, accum_out=acc[:, pi:pi + 1])
                last = (pi == NP_ - 1)
                for (pj, m, lo, hi) in mms:
                    if pj != pi:
                        continue
                    sv = 16 * m + (256 if eng == "A" else 0)
                    nc.tensor.matmul(
                        psC[0:16, lo - 512 * m:hi - 512 * m],
                        stoh[:, sv:sv + 16], t[:, lo - plo:hi - plo],
                        start=False,
                        stop=(last and (pj, m, lo, hi) == mms[-1]))

            nc.vector.tensor_copy(csb[:], psC[:])
            nc.sync.dma_start(cols_out[:], csb[:])
            nc.sync.dma_start(acc_out[:], acc[:])
    if not nc.is_finalized():
        nc.finalize()
    return nc


# ---------------------------------------------------------- host side ----
def _sorted_q(preds, labels):
    labels32 = np.asarray(labels, dtype=np.float32)
    perm = np.argsort(labels32, kind="stable")
    q16 = np.asarray(preds, dtype=np.float32)[perm].astype(np.float16)
    return q16, q16.astype(np.float64)


def make_in_maps(preds, labels):
    preds = np.asarray(preds, dtype=np.float32)
    labels = np.asarray(labels, dtype=np.float32)
    q16, qd = _sorted_q(preds, labels)
    stoh = np.zeros(544, dtype=np.float16)
    for m in range(16):
        stoh[16 * m + m] = 1.0          # DVE chunks: weight 1.0
        stoh[256 + 16 * m + m] = 0.5    # ACT sign chunks: weight 0.5
    # stoh[512:544] stays 0: zero-stationary for PE warmup
    in_maps = []
    for c in range(NCORES):
        R = _core_rowtiles(c)
        i_of_m = np.concatenate([128 * r + np.arange(128) for r in R])
        qip = np.ascontiguousarray(
            (qd[i_of_m] + MARGIN).reshape(8, 128).T.astype(np.float32))
        rows = slice(c * RPC, (c + 1) * RPC)
        in_maps.append({
            "qj": q16,
            "qip": qip,
            "stoh": stoh,
            "prow": np.ascontiguousarray(preds[rows].reshape(8, 128).T),
            "lrow": np.ascontiguousarray(labels[rows].reshape(8, 128).T),
        })
    return in_maps


def combine(results, preds, labels):
    """Fold device partials into the scalar loss (host, f64, exact)."""
    preds64 = np.asarray(preds, dtype=np.float64)
    labels64 = np.asarray(labels, dtype=np.float64)
    _, qd = _sorted_q(preds, labels)

    rows = np.zeros(N)
    cols = np.zeros(N)
    msesum = 0.0
    for c in range(NCORES):
        res = results[c]
        R = _core_rowtiles(c)
        acc = res["acc"].astype(np.float64)
        colsum = res["colsum"].astype(np.float64)
        msesum += float(res["msesq"].astype(np.float64).sum())

        # cols decode: cell [m, off] <-> b = 512m + off
        colsc = colsum.reshape(-1).copy()
        colsc[:128] = 0.0                          # b < 128: never covered
        nact = np.zeros(N)
        for (k, eng, plo, phi) in PIECES:
            if eng == "A":
                nact[plo:phi] += 64.0              # sign tiles wrote X - 0.5
        colsc[128:] += nact[128:]
        cols += colsc

        for k in range(8):
            r = R[k]
            w = WSTART[k]
            apos = 128 * r + np.arange(128)
            qa = qd[apos]
            radd = np.zeros(128)
            for pi, (kk, eng, plo, phi) in enumerate(PIECES):
                if kk != k:
                    continue
                if eng == "V":
                    radd += acc[:, pi]
                else:
                    radd += (acc[:, pi] + (phi - plo)) / 2.0
            # pollution: device also counted b with pos(b) <= pos(a)
            hi = 128 * (r + 1)
            if hi > w:
                win = np.arange(w, hi)
                qb = qd[win]
                lt = (qb[None, :] < qa[:, None] + MARGIN)
                eq = (qb[None, :] == qa[:, None] + MARGIN)
                posmask = (win[None, :] <= apos[:, None])
                actseg = np.zeros(hi - w, dtype=bool)
                for (kk, eng, plo, phi) in PIECES:
                    if kk == k and eng == "A":
                        lo_i, hi_i = max(plo - w, 0), min(phi, hi) - w
                        if hi_i > lo_i:
                            actseg[lo_i:hi_i] = True
                dveseg = ~actseg
                pv = (lt & posmask & dveseg[None, :]).sum(1)
                pa = ((lt & posmask & actseg[None, :]).sum(1)
                      + 0.5 * (eq & posmask & actseg[None, :]).sum(1))
                radd = radd - pv - pa
                cv = (lt & posmask & dveseg[None, :]).sum(0)
                ca = ((lt & posmask & actseg[None, :]).sum(0)
                      + 0.5 * (eq & posmask & actseg[None, :]).sum(0))
                np.add.at(cols, win, -(cv + ca))
            rows[apos] += radd

    # host-exact diagonal tiles {8k} (not covered by any window)
    for t in range(0, 64, 8):
        qa = qd[128 * t:128 * (t + 1)]
        X = (qa[None, :] < qa[:, None] + MARGIN)
        X &= np.triu(np.ones((128, 128), dtype=bool), k=1)
        rows[128 * t:128 * (t + 1)] += X.sum(1)
        cols[128 * t:128 * (t + 1)] += X.sum(0)

    grad = rows - cols
    TOT = rows.sum()
    ranking = MARGIN * TOT + qd @ grad
    g2 = np.sqrt((grad * grad).sum())
    mse = msesum / N
    g1 = 2.0 * np.sqrt(msesum) / N
    return np.float32(mse + g1 / (g2 + EPS) * ranking)


# ------------------------------------------------- numpy device model ----
def _sim_outputs(preds, labels):
    """Produce the same outputs the device would (for offline validation)."""
    preds = np.asarray(preds, dtype=np.float32)
    labels = np.asarray(labels, dtype=np.float32)
    q16, qd = _sorted_q(preds, labels)
    out = []
    for c in range(NCORES):
        R = _core_rowtiles(c)
        acc = np.zeros((128, NP_))
        colsum = np.zeros((16, 512))
        for pi, (k, eng, plo, phi) in enumerate(PIECES):
            r = R[k]
            qa = qd[128 * r:128 * (r + 1)]
            if eng == "V":
                X = (qd[None, plo:phi] < qa[:, None] + MARGIN).astype(np.float64)
                acc[:, pi] = X.sum(1)
                wgt, T = 1.0, X
            else:
                sgn = np.sign(qa[:, None] + MARGIN - qd[None, plo:phi])
                acc[:, pi] = sgn.sum(1)
                wgt, T = 0.5, sgn
            for m in range(plo // 512, (phi + 511) // 512):
                lo, hi = max(plo, 512 * m), min(phi, 512 * (m + 1))
                if lo < hi:
                    colsum[m, lo - 512 * m:hi - 512 * m] += \
                        wgt * T[:, lo - plo:hi - plo].sum(0)
        rows = slice(c * RPC, (c + 1) * RPC)
        d = preds[rows].astype(np.float64) - labels[rows].astype(np.float64)
        msesq = d.reshape(8, 128).T
        out.append({
            "acc": acc.astype(np.float32),
            "colsum": colsum.astype(np.float32),
            "msesq": (msesq * msesq).sum(1, keepdims=True).astype(np.float32),
        })
    return out


# ------------------------------------------------------------- driver ----
def kernel(preds, labels):
    global LAST_RESULTS
    from concourse.bass_utils import run_bass_kernel_spmd

    if "nc" not in _CACHE:
        _CACHE["nc"] = build_nc()
    in_maps = make_in_maps(preds, labels)
    res = run_bass_kernel_spmd(_CACHE["nc"], in_maps, list(range(NCORES)))
    LAST_RESULTS = res
    return combine(res.results, preds, labels)


# revision 34
# speedup vs baseline: 1.1705x; 1.0016x over previous
"""Trainium2 Bass kernel for nn_BatchRankingMSE_Loss (N=8192, 8 cores).

Reformulation: sort by labels on host (a pure data permutation). With q =
label-sorted preds, define for every pair a<b (sorted positions) the strict
indicator X(a,b) = 1{q_b < q_a + M}. Then
  ranking          = M*TOT + sum_a q_a*rows_a - sum_b q_b*cols_b
  grad_a (ranking) = rows_a - cols_a,   TOT = sum(X)
with rows/cols the row/column sums of X. The device evaluates the X grid
once and reduces it both ways:

Per core (SPMD, identical program; core c owns row-tiles
R_c = {8k + (c+k)%8}): 8 big flip-layout ops, op k =
[128 partitions = a-values of tile R_c[k]] x [free b in [128*(8k+1), 8192)].
Each op is split into DVE pieces (tensor_scalar is_lt with add-reduce
accum) and ACT pieces (Sign activation with accum), sub-split at the qj
half-DMA boundary so compute starts on the first half:
  - accum_out (free-axis sum)  -> row sums
  - the out tiles, streamed through TensorE with one-hot stationary columns
    into a single [16, 512] PSUM tile -> column sums (chunk m of 512 b's
    accumulates into PSUM row m; ACT sign tiles use 0.5-valued stationary)
Zero-stationary warmup matmuls keep the PE HAM busy during the DMA wait.
Window overshoot (b at/below own position) and the 8 uncovered diagonal
tiles {8k} are corrected exactly on host; mse partials also on device.
"""

import numpy as np

MARGIN = 2.0
EPS = 1e-4
N = 8192
NCORES = 8
RPC = N // NCORES        # rows per core = 1024
ACT_ENABLE = True
HALF = 4096              # qj DMA half boundary

_CACHE = {}
LAST_RESULTS = None      # test.py introspects timing from here


# ---------------------------------------------------------------- plan ----
def _core_rowtiles(c):
    return [8 * k + (c + k) % 8 for k in range(8)]


WSTART = [128 * (8 * k + 1) for k in range(8)]     # flip-op window starts


def _make_pieces(act_enable=ACT_ENABLE):
    """Partition each op-k window into engine pieces (uniform across cores).

    Returns list of (k, engine, lo, hi) with 512-aligned boundaries, also
    split at HALF so early pieces only need the first qj half.
    """
    cV = lambda fd: 430 + 1.042 * fd
    cA = lambda fd: 1150 + 0.833 * fd
    dve = 2000.0          # mse + psum copy + slack
    act = 1400.0          # Sign table load
    pieces = []
    # choose ACT suffix split per op by greedy balance
    for k in sorted(range(8), key=lambda k: WSTART[k]):
        w = WSTART[k]
        fd = N - w
        best = None
        if not act_enable:
            best = (0, N, dve + cV(fd), act)
        else:
            for s in range(w, N + 1, 512):
                fv, fa = s - w, N - s
                d2 = dve + (cV(fv) if fv else 0)
                a2 = act + (cA(fa) if fa else 0)
                m = max(d2, a2)
                if best is None or m < best[0]:
                    best = (m, s, d2, a2)
        _, s, dve, act = best
        for (eng, lo, hi) in (("V", w, s), ("A", s, N)):
            if lo >= hi:
                continue
            cuts = [b for b in (2048, HALF) if lo < b < hi]
            for a, b in zip([lo] + cuts, cuts + [hi]):
                pieces.append((k, eng, a, b))
    # order: by qj-slice arrival (quarter of the start), then size desc
    pieces.sort(key=lambda p: (p[2] // 2048, -(p[3] - p[2])))
    return pieces


PIECES = _make_pieces()
NP_ = len(PIECES)


# ------------------------------------------------------------- program ----
def build_nc():
    import concourse.bass as bass
    import concourse.mybir as mybir
    from concourse import bacc, tile

    dt = mybir.dt
    Af = mybir.ActivationFunctionType
    Op = mybir.AluOpType

    nc = bacc.Bacc(None)
    qj_in = nc.dram_tensor("qj", [128, N], dt.float16, kind="ExternalInput")
    qip_in = nc.dram_tensor("qip", [128, 8], dt.float32, kind="ExternalInput")
    stoh_in = nc.dram_tensor("stoh", [544], dt.float16, kind="ExternalInput")
    prow_in = nc.dram_tensor("prow", [128, 8], dt.float32, kind="ExternalInput")
    lrow_in = nc.dram_tensor("lrow", [128, 8], dt.float32, kind="ExternalInput")
    acc_out = nc.dram_tensor("acc", [128, NP_], dt.float32, kind="ExternalOutput")
    cols_out = nc.dram_tensor("colsum", [16, 512], dt.float32, kind="ExternalOutput")
    mse_out = nc.dram_tensor("msesq", [128, 1], dt.float32, kind="ExternalOutput")

    # PE chunk-matmuls per piece: (piece idx, chunk m, lo, hi)
    mms = []
    for pi, (k, eng, plo, phi) in enumerate(PIECES):
        for m in range(plo // 512, (phi + 511) // 512):
            lo, hi = max(plo, 512 * m), min(phi, 512 * (m + 1))
            if lo < hi:
                mms.append((pi, m, lo, hi))
    NWARM = 14

    with tile.TileContext(nc) as tc:
        with (
            tc.tile_pool(name="persist", bufs=1) as pp,
            tc.tile_pool(name="work", bufs=4) as wp,
            tc.tile_pool(name="psum", bufs=1, space="PSUM") as qp,
        ):
            qj = pp.tile([128, N], dt.float16)
            qip = pp.tile([128, 8], dt.float32)
            stoh = pp.tile([128, 544], dt.float16)
            acc = pp.tile([128, NP_], dt.float32)
            msea = pp.tile([128, 1], dt.float32)
            csb = pp.tile([16, 512], dt.float32)
            pr = pp.tile([128, 8], dt.float32)
            lr = pp.tile([128, 8], dt.float32)
            dmse = pp.tile([128, 8], dt.float32)
            sqms = pp.tile([128, 8], dt.float32)

            psC = qp.tile([16, 512], dt.float32, tag="psc", name="psc")

            # qj owns the Sync queue from instruction zero; the small inputs
            # issue concurrently from the (still idle) Scalar queue
            for s in range(4):
                cs = slice(s * 2048, (s + 1) * 2048)
                nc.sync.dma_start(qj[:, cs], qj_in[:, cs])
            nc.scalar.dma_start(stoh[:], stoh_in[:].partition_broadcast(128))
            nc.scalar.dma_start(qip[:], qip_in[:])
            nc.scalar.dma_start(pr[:], prow_in[:])
            nc.scalar.dma_start(lr[:], lrow_in[:])

            # load the Sign table while DMAs stream (dummy op on stoh)
            dumm = pp.tile([128, 16], dt.float16)
            nc.scalar.activation(dumm[:], stoh[:, 0:16],
                                 Af.Sign, bias=0.0, scale=1.0)

            # PE warmup: zero-stationary matmuls (add 0 into psC) to lift
            # the HAM clock gate while qj streams in. First one clears psC.
            for wi in range(NWARM):
                nc.tensor.matmul(psC[0:16, 0:512], stoh[:, 512:528],
                                 stoh[:, 0:512], start=(wi == 0), stop=False)

            # mse partials: sum_free (p-l)^2 per partition
            nc.vector.scalar_tensor_tensor(
                dmse[:], pr[:], 0.0, lr[:], op0=Op.add, op1=Op.subtract)
            nc.vector.scalar_tensor_tensor(
                sqms[:], dmse[:], 1.0, dmse[:], op0=Op.mult, op1=Op.mult,
                accum_out=msea[:])
            nc.sync.dma_start(mse_out[:], msea[:])

            for pi, (k, eng, plo, phi) in enumerate(PIECES):
                fd = phi - plo
                t = wp.tile([128, fd], dt.float16, tag=eng)
                if eng == "V":
                    # X = 1{q_b < q_a + M}; op1/scalar2 = add-reduce to accum
                    nc.vector.tensor_scalar(
                        t[:], qj[:, plo:phi], qip[:, k:k + 1], 0.0,
                        op0=Op.is_lt, op1=Op.add,
                        accum_out=acc[:, pi:pi + 1])
                else:
                    nc.scalar.activation(
                        t[:], qj[:, plo:phi], Af.Sign, bias=qip[:, k:k + 1],
                        scale=-1.0, accum_out=acc[:, pi:pi + 1])
                last = (pi == NP_ - 1)
                for (pj, m, lo, hi) in mms:
                    if pj != pi:
                        continue
                    sv = 16 * m + (256 if eng == "A" else 0)
                    nc.tensor.matmul(
                        psC[0:16, lo - 512 * m:hi - 512 * m],
                        stoh[:, sv:sv + 16], t[:, lo - plo:hi - plo],
                        start=False,
                        stop=(last and (pj, m, lo, hi) == mms[-1]))

            nc.vector.tensor_copy(csb[:], psC[:])
            nc.sync.dma_start(cols_out[:], csb[:])
            nc.sync.dma_start(acc_out[:], acc[:])
    if not nc.is_finalized():
        nc.finalize()
    return nc


# ---------------------------------------------------------- host side ----
def _sorted_q(preds, labels):
    labels32 = np.asarray(labels, dtype=np.float32)
    perm = np.argsort(labels32, kind="stable")
    q16 = np.asarray(preds, dtype=np.float32)[perm].astype(np.float16)
    return q16, q16.astype(np.float64)


def make_in_maps(preds, labels):
    preds = np.asarray(preds, dtype=np.float32)
    labels = np.asarray(labels, dtype=np.float32)
    q16, qd = _sorted_q(preds, labels)
    stoh = np.zeros(544, dtype=np.float16)
    for m in range(16):
        stoh[16 * m + m] = 1.0          # DVE chunks: weight 1.0
        stoh[256 + 16 * m + m] = 0.5    # ACT sign chunks: weight 0.5
    # stoh[512:544] stays 0: zero-stationary for PE warmup
    qjrep = np.ascontiguousarray(np.broadcast_to(q16, (128, N)))
    in_maps = []
    for c in range(NCORES):
        R = _core_rowtiles(c)
        i_of_m = np.concatenate([128 * r + np.arange(128) for r in R])
        qip = np.ascontiguousarray(
            (qd[i_of_m] + MARGIN).reshape(8, 128).T.astype(np.float32))
        rows = slice(c * RPC, (c + 1) * RPC)
        in_maps.append({
            "qj": qjrep,
            "qip": qip,
            "stoh": stoh,
            "prow": np.ascontiguousarray(preds[rows].reshape(8, 128).T),
            "lrow": np.ascontiguousarray(labels[rows].reshape(8, 128).T),
        })
    return in_maps


def combine(results, preds, labels):
    """Fold device partials into the scalar loss (host, f64, exact)."""
    preds64 = np.asarray(preds, dtype=np.float64)
    labels64 = np.asarray(labels, dtype=np.float64)
    _, qd = _sorted_q(preds, labels)

    rows = np.zeros(N)
    cols = np.zeros(N)
    msesum = 0.0
    for c in range(NCORES):
        res = results[c]
        R = _core_rowtiles(c)
        acc = res["acc"].astype(np.float64)
        colsum = res["colsum"].astype(np.float64)
        msesum += float(res["msesq"].astype(np.float64).sum())

        # cols decode: cell [m, off] <-> b = 512m + off
        colsc = colsum.reshape(-1).copy()
        colsc[:128] = 0.0                          # b < 128: never covered
        nact = np.zeros(N)
        for (k, eng, plo, phi) in PIECES:
            if eng == "A":
                nact[plo:phi] += 64.0              # sign tiles wrote X - 0.5
        colsc[128:] += nact[128:]
        cols += colsc

        for k in range(8):
            r = R[k]
            w = WSTART[k]
            apos = 128 * r + np.arange(128)
            qa = qd[apos]
            radd = np.zeros(128)
            for pi, (kk, eng, plo, phi) in enumerate(PIECES):
                if kk != k:
                    continue
                if eng == "V":
                    radd += acc[:, pi]
                else:
                    radd += (acc[:, pi] + (phi - plo)) / 2.0
            # pollution: device also counted b with pos(b) <= pos(a)
            hi = 128 * (r + 1)
            if hi > w:
                win = np.arange(w, hi)
                qb = qd[win]
                lt = (qb[None, :] < qa[:, None] + MARGIN)
                eq = (qb[None, :] == qa[:, None] + MARGIN)
                posmask = (win[None, :] <= apos[:, None])
                actseg = np.zeros(hi - w, dtype=bool)
                for (kk, eng, plo, phi) in PIECES:
                    if kk == k and eng == "A":
                        lo_i, hi_i = max(plo - w, 0), min(phi, hi) - w
                        if hi_i > lo_i:
                            actseg[lo_i:hi_i] = True
                dveseg = ~actseg
                pv = (lt & posmask & dveseg[None, :]).sum(1)
                pa = ((lt & posmask & actseg[None, :]).sum(1)
                      + 0.5 * (eq & posmask & actseg[None, :]).sum(1))
                radd = radd - pv - pa
                cv = (lt & posmask & dveseg[None, :]).sum(0)
                ca = ((lt & posmask & actseg[None, :]).sum(0)
                      + 0.5 * (eq & posmask & actseg[None, :]).sum(0))
                np.add.at(cols, win, -(cv + ca))
            rows[apos] += radd

    # host-exact diagonal tiles {8k} (not covered by any window)
    for t in range(0, 64, 8):
        qa = qd[128 * t:128 * (t + 1)]
        X = (qa[None, :] < qa[:, None] + MARGIN)
        X &= np.triu(np.ones((128, 128), dtype=bool), k=1)
        rows[128 * t:128 * (t + 1)] += X.sum(1)
        cols[128 * t:128 * (t + 1)] += X.sum(0)

    grad = rows - cols
    TOT = rows.sum()
    ranking = MARGIN * TOT + qd @ grad
    g2 = np.sqrt((grad * grad).sum())
    mse = msesum / N
    g1 = 2.0 * np.sqrt(msesum) / N
    return np.float32(mse + g1 / (g2 + EPS) * ranking)


# ------------------------------------------------- numpy device model ----
def _sim_outputs(preds, labels):
    """Produce the same outputs the device would (for offline validation)."""
    preds = np.asarray(preds, dtype=np.float32)
    labels = np.asarray(labels, dtype=np.float32)
    q16, qd = _sorted_q(preds, labels)
    out = []
    for c in range(NCORES):
        R = _core_rowtiles(c)
        acc = np.zeros((128, NP_))
        colsum = np.zeros((16, 512))
        for pi, (k, eng, plo, phi) in enumerate(PIECES):
            r = R[k]
            qa = qd[128 * r:128 * (r + 1)]
            if eng == "V":
                X = (qd[None, plo:phi] < qa[:, None] + MARGIN).astype(np.float64)
                acc[:, pi] = X.sum(1)
                wgt, T = 1.0, X
            else:
                sgn = np.sign(qa[:, None] + MARGIN - qd[None, plo:phi])
                acc[:, pi] = sgn.sum(1)
                wgt, T = 0.5, sgn
            for m in range(plo // 512, (phi + 511) // 512):
                lo, hi = max(plo, 512 * m), min(phi, 512 * (m + 1))
                if lo < hi:
                    colsum[m, lo - 512 * m:hi - 512 * m] += \
                        wgt * T[:, lo - plo:hi - plo].sum(0)
        rows = slice(c * RPC, (c + 1) * RPC)
        d = preds[rows].astype(np.float64) - labels[rows].astype(np.float64)
        msesq = d.reshape(8, 128).T
        out.append({
            "acc": acc.astype(np.float32),
            "colsum": colsum.astype(np.float32),
            "msesq": (msesq * msesq).sum(1, keepdims=True).astype(np.float32),
        })
    return out


# ------------------------------------------------------------- driver ----
def kernel(preds, labels):
    global LAST_RESULTS
    from concourse.bass_utils import run_bass_kernel_spmd

    if "nc" not in _CACHE:
        _CACHE["nc"] = build_nc()
    in_maps = make_in_maps(preds, labels)
    res = run_bass_kernel_spmd(_CACHE["nc"], in_maps, list(range(NCORES)))
    LAST_RESULTS = res
    return combine(res.results, preds, labels)
